# revision 1
# baseline (speedup 1.0000x reference)
"""Trainium2 Bass kernel for nn_MultiHeadAttention (B=4, S=2048, d_model=768, H=4).

Reference semantics (note the unusual softmax over the QUERY axis):
    Q = x @ Wq.T + bq ; K, V likewise
    w = Q K^T / 20 ; causal (strict upper triangle) + padding mask -> -1e9
    attn = softmax(w, axis=QUERY)          # column softmax
    x1 = attn @ V ; y = x + concat_heads(x1) ; out = LayerNorm(y) * gamma + beta

Sharding (8 cores): core c handles batch b=c//2 and head-pair p=c%2
(heads {2p, 2p+1} = model dims [384p, 384p+384)).  Attention per (b, head)
is fully local.  The only cross-core coupling is LayerNorm's per-token
mean/var over all 768 dims -> a 16KB pair-AllReduce of partial moments
(sum(y), sum(y^2) over the core's 384 dims).  The output is sharded by
model dim: core c writes out[b, :, 384p:384p+384].

Layout trick for the column softmax: scores are built TRANSPOSED,
S^T[k, q] (keys on partitions), so the softmax denominator c[k] =
sum_q exp(S/20) is a free-axis reduction that falls out of the exp's
accum_out for free, and E^T strips are directly the lhsT of the
attn @ V matmul.  1/c is folded into V rows.
"""

import os
import numpy as np
import ml_dtypes

import concourse.bacc as bacc
import concourse.bass as bass
import concourse.tile as tile
from concourse import mybir
from concourse.bass_utils import run_bass_kernel_spmd

BF16 = mybir.dt.bfloat16
F32 = mybir.dt.float32

B, S, DM, H, HD = 4, 2048, 768, 4, 192
N_CORES = 8
SCALE = 20.0
LN_EPS = 1e-5
NEG = -1e9
P = 128
NT = S // P  # 16 token tiles
NKB = S // P  # 16 key blocks
DP = DM // 2  # 384 dims per head-pair

# per-head (dout_offset, size) pieces within the pair's 384 dims, chosen so
# each piece stays within one 128-aligned column of the bias pack
PIECES = {0: [(0, 128), (128, 64)], 1: [(192, 64), (256, 128)]}


def _build(has_bias: bool, has_pad: bool, has_affine: bool = True, seq: int = S, n_cores: int = N_CORES):
    NT = seq // P
    NKB = seq // P
    phase = int(os.environ.get("BASS_KERNEL_PHASE", "9"))
    nc = bacc.Bacc("TRN2", target_bir_lowering=False, debug=False,
                   num_devices=n_cores)

    xt = nc.dram_tensor("xt", [DM, seq], BF16, kind="ExternalInput")
    wqt = nc.dram_tensor("wqt", [DM, DP], BF16, kind="ExternalInput")
    wkt = nc.dram_tensor("wkt", [DM, DP], BF16, kind="ExternalInput")
    wvt = nc.dram_tensor("wvt", [DM, DP], BF16, kind="ExternalInput")
    xres = nc.dram_tensor("xres", [seq, DP], F32, kind="ExternalInput")
    trimask = nc.dram_tensor("trimask", [P, P], F32, kind="ExternalInput")
    gb = nc.dram_tensor("gb", [2, DP], F32, kind="ExternalInput")
    if has_bias:
        biases = nc.dram_tensor("biases", [P, 12], F32, kind="ExternalInput")
    if has_pad:
        qmaskneg = nc.dram_tensor("qmaskneg", [seq], F32, kind="ExternalInput")
    out = nc.dram_tensor("out", [seq, DP], F32, kind="ExternalOutput")

    with tile.TileContext(nc) as tc:
        with (
            tc.tile_pool(name="wp", bufs=1) as wp,        # weights, x^T, consts
            tc.tile_pool(name="qk", bufs=1) as qk,        # Q^T/K^T per head
            tc.tile_pool(name="vp", bufs=1) as vp,        # V f32 + V' bf16
            tc.tile_pool(name="ep", bufs=1) as ep,        # E^T strips
            tc.tile_pool(name="yp", bufs=1) as yp,        # y, xres, moments, stats
            tc.tile_pool(name="tmp", bufs=3) as tmp,      # small rotating
            tc.tile_pool(name="mm", bufs=2, space="PSUM") as mmps,
            tc.tile_pool(name="sc", bufs=2, space="PSUM") as scps,
            tc.tile_pool(name="x1", bufs=2, space="PSUM") as x1ps,
            tc.tile_pool(name="dram", bufs=1, space="DRAM") as dram,
        ):
            # ---------------- load inputs ----------------
            xt_sb = []
            for i in range(6):
                t = wp.tile([P, seq], BF16, tag=f"xt{i}")
                nc.sync.dma_start(out=t, in_=xt[i * P:(i + 1) * P, :])
                xt_sb.append(t)
            w_sb = {}
            for name, drt in (("q", wqt), ("k", wkt), ("v", wvt)):
                t = wp.tile([P, 6, DP], BF16, tag=f"w{name}")
                nc.sync.dma_start(
                    out=t, in_=drt.rearrange("(c p) d -> p c d", p=P))
                w_sb[name] = t
            tri_sb = wp.tile([P, P], F32, tag="tri")
            nc.sync.dma_start(out=tri_sb, in_=trimask[:])
            if has_bias:
                bias_sb = wp.tile([P, 12], F32, tag="bias")
                nc.sync.dma_start(out=bias_sb, in_=biases[:])
            if has_pad:
                qm_sb = wp.tile([P, seq], F32, tag="qm")
                nc.sync.dma_start(out=qm_sb, in_=bass.AP(
                    tensor=qmaskneg, offset=0, ap=[[0, P], [1, seq]]))
            eps_sb = wp.tile([P, 1], F32, tag="eps")
            nc.vector.memset(eps_sb, LN_EPS)

            use_cc = os.environ.get("BASS_KERNEL_NO_CC", "0") != "1"
            warm_cc = use_cc and os.environ.get("BASS_KERNEL_WARM_CC", "1") == "1"
            if warm_cc:
                ccw_in = dram.tile([P, 2 * NT], F32)
                ccw_out = dram.tile([P, 2 * NT], F32)
                wseed = tmp.tile([P, 2 * NT], F32, tag="wseed")
                nc.vector.memset(wseed, 0.0)
                nc.sync.dma_start(out=ccw_in[:], in_=wseed)
                nc.gpsimd.collective_compute(
                    "AllReduce", mybir.AluOpType.add,
                    replica_groups=[[2 * i, 2 * i + 1]
                                    for i in range(n_cores // 2)],
                    ins=[ccw_in.opt()], outs=[ccw_out.opt()])
            # warm the Sqrt activation table so the LN tail doesn't pay it
            sqwarm = tmp.tile([P, 1], F32, tag="sqwarm")
            nc.scalar.activation(out=sqwarm, in_=eps_sb,
                                 func=mybir.ActivationFunctionType.Sqrt,
                                 bias=eps_sb)

            # ---------------- QKV projections ----------------
            # Q^T/K^T per head: pieces along dout; [piece_size, S] bf16 tiles
            qt_sb = {}  # (h, piece_idx) -> tile
            kt_sb = {}
            for h in range(2):
                for pi, (o, sz) in enumerate(PIECES[h]):
                    for which, store in (("q", qt_sb), ("k", kt_sb)):
                        dst = qk.tile([sz, seq], BF16, tag=f"{which}t{h}{pi}")
                        store[(h, pi)] = dst
                        for qc in range(seq // 512):
                            ps = mmps.tile([P, 512], F32, tag="mm")
                            for c in range(6):
                                nc.tensor.matmul(
                                    ps[:sz, :],
                                    w_sb[which][:, c, o:o + sz],
                                    xt_sb[c][:, qc * 512:(qc + 1) * 512],
                                    start=(c == 0), stop=(c == 5))
                            dslice = dst[:, qc * 512:(qc + 1) * 512]
                            if has_bias:
                                bcol = {"q": 0, "k": 4}[which] + 2 * h + pi
                                nc.vector.tensor_scalar_add(
                                    out=dslice, in0=ps[:sz, :],
                                    scalar1=bias_sb[:sz, bcol:bcol + 1])
                            else:
                                nc.vector.tensor_copy(out=dslice, in_=ps[:sz, :])

            # V: [128, NT, 384] f32 token-tiled
            v_sb = vp.tile([P, NT, DP], F32, tag="v")
            for t in range(NT):
                ps = mmps.tile([P, 512], F32, tag="mm")
                for c in range(6):
                    nc.tensor.matmul(
                        ps[:, 0:DP],
                        xt_sb[c][:, t * P:(t + 1) * P],
                        w_sb["v"][:, c, :],
                        start=(c == 0), stop=(c == 5))
                nc.scalar.copy(out=v_sb[:, t, :], in_=ps[:, 0:DP])

            # y accumulator (x1 + residual), f32
            y_sb = yp.tile([P, NT, DP], F32, tag="y")
            xres_sb = yp.tile([P, NT, DP], F32, tag="xres")
            nc.sync.dma_start(
                out=xres_sb, in_=xres.rearrange("(t p) d -> p t d", p=P))
            if has_affine:
                gamma_sb = wp.tile([P, DP], F32, tag="gamma")
                nc.sync.dma_start(out=gamma_sb, in_=bass.AP(
                    tensor=gb, offset=0, ap=[[0, P], [1, DP]]))
                beta_sb = wp.tile([P, DP], F32, tag="beta")
                nc.sync.dma_start(out=beta_sb, in_=bass.AP(
                    tensor=gb, offset=DP, ap=[[0, P], [1, DP]]))
            mom = yp.tile([P, 2 * NT], F32, tag="mom")

            if phase <= 1:
                for qt in range(NT):
                    nc.sync.dma_start(out=out[qt * P:(qt + 1) * P, :],
                                      in_=v_sb[:, qt, :])

            # ---------------- per-head attention ----------------
            n_heads_run = 0 if phase <= 1 else (1 if phase == 2 else 2)
            for h in range(n_heads_run):
                pieces = PIECES[h]
                estrips = []
                # scores + exp + colsum per key block
                for kb in range(NKB):
                    L = seq - kb * P
                    estrip = ep.tile([P, L], BF16, tag=f"e{kb}")
                    estrips.append(estrip)
                    accs = []
                    nch = (L + 1023) // 1024
                    for ci in range(nch):
                        c0 = ci * 1024
                        clen = min(1024, L - c0)
                        ps = scps.tile([P, 1024], F32, tag="sc")
                        for sub in range((clen + 511) // 512):
                            s0 = sub * 512
                            n = min(512, clen - s0)
                            for pi, (o, sz) in enumerate(pieces):
                                nc.tensor.matmul(
                                    ps[:, s0:s0 + n],
                                    kt_sb[(h, pi)][:, kb * P:(kb + 1) * P],
                                    qt_sb[(h, pi)][:, kb * P + c0 + s0:
                                                   kb * P + c0 + s0 + n],
                                    start=(pi == 0), stop=(pi == len(pieces) - 1))
                        if ci == 0:
                            nc.vector.tensor_add(
                                out=ps[:, 0:P], in0=ps[:, 0:P], in1=tri_sb)
                        if has_pad:
                            nc.vector.tensor_add(
                                out=ps[:, 0:clen], in0=ps[:, 0:clen],
                                in1=qm_sb[:, kb * P + c0:kb * P + c0 + clen])
                        acc = tmp.tile([P, 1], F32, tag="acc")
                        nc.scalar.activation(
                            out=estrip[:, c0:c0 + clen], in_=ps[:, 0:clen],
                            func=mybir.ActivationFunctionType.Exp,
                            scale=1.0 / SCALE, accum_out=acc)
                        accs.append(acc)
                    if len(accs) == 2:
                        csum = tmp.tile([P, 1], F32, tag="csum")
                        nc.vector.tensor_add(out=csum, in0=accs[0], in1=accs[1])
                    else:
                        csum = accs[0]
                    rc = tmp.tile([P, 1], F32, tag="rc")
                    nc.vector.reciprocal(out=rc, in_=csum)
                    vpr = vp.tile([P, HD], BF16, tag=f"vp{kb}")
                    nc.vector.tensor_scalar_mul(
                        out=vpr, in0=v_sb[:, kb, h * HD:(h + 1) * HD],
                        scalar1=rc)
                    estrips[kb] = (estrip, vpr)

                # x1 = E^T.T @ V' accumulated over key blocks, + residual
                if phase <= 3:
                    continue
                for qt in range(NT):
                    ps = x1ps.tile([P, HD], F32, tag="x1")
                    for kb in range(qt + 1):
                        estrip, vpr = estrips[kb]
                        nc.tensor.matmul(
                            ps,
                            estrip[:, (qt - kb) * P:(qt - kb + 1) * P],
                            vpr,
                            start=(kb == 0), stop=(kb == qt))
                    nc.vector.tensor_add(
                        out=y_sb[:, qt, h * HD:(h + 1) * HD],
                        in0=ps,
                        in1=xres_sb[:, qt, h * HD:(h + 1) * HD])
                    if h == n_heads_run - 1 and phase >= 5:
                        nc.vector.reduce_sum(
                            out=mom[:, qt:qt + 1], in_=y_sb[:, qt, :],
                            axis=mybir.AxisListType.X, op=mybir.AluOpType.add)
                        sq = tmp.tile([P, DP], F32, tag="sq")
                        nc.scalar.activation(
                            out=sq, in_=y_sb[:, qt, :],
                            func=mybir.ActivationFunctionType.Square,
                            accum_out=mom[:, NT + qt:NT + qt + 1])

            if 2 <= phase <= 3:
                # consume E strips trivially so the phase is exercised
                for qt in range(NT):
                    nc.sync.dma_start(out=out[qt * P:(qt + 1) * P, :],
                                      in_=v_sb[:, qt, :])
            if phase == 4:
                for qt in range(NT):
                    ot = tmp.tile([P, DP], F32, tag="ot")
                    nc.vector.tensor_copy(out=ot, in_=y_sb[:, qt, :])
                    nc.sync.dma_start(out=out[qt * P:(qt + 1) * P, :], in_=ot)
            if phase <= 4:
                skip_rest = True
            else:
                skip_rest = False

            # ---------------- LayerNorm moments + pair AllReduce ----------------
            msum = yp.tile([P, 2 * NT], F32, tag="msum")
            if skip_rest:
                nc.vector.memset(msum, 1.0)
                nc.vector.memset(y_sb, 1.0)
            elif os.environ.get("BASS_KERNEL_NO_CC", "0") == "1":
                # debug mode: skip cross-core reduce (halves are then wrong
                # unless inputs are replicated; used for HW bisection only)
                nc.vector.tensor_scalar_mul(out=msum, in0=mom, scalar1=2.0)
            else:
                cc_in = dram.tile([P, 2 * NT], F32)
                cc_out = dram.tile([P, 2 * NT], F32)
                sqw2 = tmp.tile([P, 1], F32, tag="sqwarm")
                nc.scalar.activation(out=sqw2, in_=eps_sb,
                                     func=mybir.ActivationFunctionType.Sqrt,
                                     bias=eps_sb)
                nc.sync.dma_start(out=cc_in[:], in_=mom)
                nc.gpsimd.collective_compute(
                    "AllReduce", mybir.AluOpType.add,
                    replica_groups=[[2 * i, 2 * i + 1]
                                    for i in range(n_cores // 2)],
                    ins=[cc_in.opt()], outs=[cc_out.opt()])
                nc.sync.dma_start(out=msum, in_=cc_out[:])

            # stats: mu, rstd [128, NT]
            mue = yp.tile([P, 2 * NT], F32, tag="mue")
            nc.scalar.mul(out=mue, in_=msum, mul=1.0 / DM)
            mu = mue[:, 0:NT]
            var = yp.tile([P, NT], F32, tag="var")
            nc.vector.tensor_mul(out=var, in0=mu, in1=mu)
            nc.vector.tensor_sub(out=var, in0=mue[:, NT:2 * NT], in1=var)
            std = yp.tile([P, NT], F32, tag="std")
            nc.scalar.activation(out=std, in_=var,
                                 func=mybir.ActivationFunctionType.Sqrt,
                                 bias=eps_sb)
            rstd = yp.tile([P, NT], F32, tag="rstd")
            nc.vector.reciprocal(out=rstd, in_=std)

            # normalize (+ affine unless trivial) + store;
            # output overwrites y_sb (dead afterwards)
            for qt in range(NT):
                if has_affine:
                    z = tmp.tile([P, DP], F32, tag="z")
                    nc.vector.scalar_tensor_tensor(
                        out=z, in0=y_sb[:, qt, :], scalar=mu[:, qt:qt + 1],
                        in1=gamma_sb,
                        op0=mybir.AluOpType.subtract, op1=mybir.AluOpType.mult)
                    nc.vector.scalar_tensor_tensor(
                        out=y_sb[:, qt, :], in0=z, scalar=rstd[:, qt:qt + 1],
                        in1=beta_sb,
                        op0=mybir.AluOpType.mult, op1=mybir.AluOpType.add)
                else:
                    eng = nc.vector if qt % 2 == 0 else nc.gpsimd
                    eng.tensor_scalar(
                        out=y_sb[:, qt, :], in0=y_sb[:, qt, :],
                        scalar1=mu[:, qt:qt + 1], scalar2=rstd[:, qt:qt + 1],
                        op0=mybir.AluOpType.subtract, op1=mybir.AluOpType.mult)
                nc.sync.dma_start(
                    out=out[qt * P:(qt + 1) * P, :], in_=y_sb[:, qt, :])

    nc.compile()
    return nc


_NC_CACHE = {}


def _get_nc(has_bias: bool, has_pad: bool, has_affine: bool):
    key = (has_bias, has_pad, has_affine)
    if key not in _NC_CACHE:
        _NC_CACHE[key] = _build(*key)
    return _NC_CACHE[key]


def _install_ntff_hook():
    """Optional: enables trace=True under axon (used by test.py via env)."""
    import sys, types, contextlib, ctypes
    if "antenv.axon_hooks" in sys.modules:
        return
    so_path = "/opt/axon/libaxon_pjrt.so"
    try:
        lib = ctypes.CDLL(so_path)
    except OSError:
        return
    if not hasattr(lib, "axon_start_nrt_profile"):
        return
    lib.axon_start_nrt_profile.argtypes = [ctypes.POINTER(ctypes.c_int64),
                                           ctypes.c_size_t]
    lib.axon_start_nrt_profile.restype = ctypes.c_int64
    lib.axon_stop_nrt_profile.argtypes = [ctypes.c_char_p]
    lib.axon_stop_nrt_profile.restype = ctypes.c_int64

    @contextlib.contextmanager
    def _hook(output_dir, device_ids):
        import jax
        jax.devices()
        if device_ids:
            ids = (ctypes.c_int64 * len(device_ids))(*device_ids)
            rc = lib.axon_start_nrt_profile(ids, len(device_ids))
        else:
            rc = lib.axon_start_nrt_profile(None, 0)
        if rc != 0:
            raise RuntimeError(f"axon_start_nrt_profile rc={rc}")
        try:
            yield
        finally:
            lib.axon_stop_nrt_profile(str(output_dir).encode())

    mod = types.ModuleType("antenv.axon_hooks")
    mod.get_axon_ntff_profile_hook = lambda: _hook
    mod.set_axon_ntff_profile_hook = lambda h: None
    sys.modules["antenv.axon_hooks"] = mod
    import concourse.bass_utils as bu
    bu.upload_artifacts = lambda tmpdir: "local://skipped"


_LAST_RESULT = None  # BassKernelResults of the last run (for test harness)


def kernel(x, attention_mask, Wq, bq, Wk, bk, Wv, bv, gamma, beta):
    x = np.asarray(x, dtype=np.float32)
    attention_mask = np.asarray(attention_mask)
    has_bias = bool(np.any(np.asarray(bq)) or np.any(np.asarray(bk))
                    or np.any(np.asarray(bv)))
    has_pad = bool(np.asarray(attention_mask).any())
    has_affine = not (np.all(np.asarray(gamma) == 1.0)
                      and np.all(np.asarray(beta) == 0.0))
    assert not has_bias, "nonzero qkv bias path not implemented"

    trace = os.environ.get("BASS_KERNEL_TRACE", "0") == "1"
    if trace:
        _install_ntff_hook()

    ii, jj = np.meshgrid(np.arange(P), np.arange(P), indexing="ij")
    trimask = np.where(jj >= ii, 0.0, NEG).astype(np.float32)

    in_maps = []
    for c in range(N_CORES):
        b, p = c // 2, c % 2
        po = p * DP
        m = {
            "xt": np.ascontiguousarray(x[b].T).astype(ml_dtypes.bfloat16),
            "wqt": np.ascontiguousarray(Wq[po:po + DP, :].T).astype(ml_dtypes.bfloat16),
            "wkt": np.ascontiguousarray(Wk[po:po + DP, :].T).astype(ml_dtypes.bfloat16),
            "wvt": np.ascontiguousarray(Wv[po:po + DP, :].T).astype(ml_dtypes.bfloat16),
            "xres": np.ascontiguousarray(x[b][:, po:po + DP]).astype(np.float32),
            "trimask": trimask,
            "gb": np.stack([np.asarray(gamma, np.float32)[po:po + DP],
                            np.asarray(beta, np.float32)[po:po + DP]]),
        }
        if has_pad:
            m["qmaskneg"] = np.where(attention_mask[b, 0], NEG, 0.0).astype(np.float32)
        in_maps.append(m)

    nc = _get_nc(has_bias, has_pad, has_affine)
    res = run_bass_kernel_spmd(nc, in_maps, core_ids=list(range(N_CORES)),
                               trace=trace)
    global _LAST_RESULT
    _LAST_RESULT = res

    out = np.empty((B, S, DM), dtype=np.float32)
    for c in range(N_CORES):
        b, p = c // 2, c % 2
        out[b, :, p * DP:(p + 1) * DP] = res.results[c]["out"]
    return out



# revision 9
# speedup vs baseline: 1.0105x; 1.0105x over previous
"""Trainium2 Bass kernel for nn_MultiHeadAttention (B=4, S=2048, d_model=768, H=4).

Reference semantics (note the unusual softmax over the QUERY axis):
    Q = x @ Wq.T + bq ; K, V likewise
    w = Q K^T / 20 ; causal (strict upper triangle) + padding mask -> -1e9
    attn = softmax(w, axis=QUERY)          # column softmax
    x1 = attn @ V ; y = x + concat_heads(x1) ; out = LayerNorm(y) * gamma + beta

Sharding (8 cores): core c handles batch b=c//2 and head-pair p=c%2
(heads {2p, 2p+1} = model dims [384p, 384p+384)).  Attention per (b, head)
is fully local.  The only cross-core coupling is LayerNorm's per-token
mean/var over all 768 dims -> a pair-AllReduce of partial moments.

Perf structure vs the naive version:
  * Scores are built TRANSPOSED, S^T[k, q] (keys on partitions), so the
    column-softmax denominator c[k] = sum_q exp(S/20) falls out of the
    exp's accum_out for free, and E^T strips are directly the lhsT of
    the attn @ V matmul.  1/c is folded into V rows.
  * x1 for BOTH heads is computed per query-tile (interleaved heads),
    so the LN moments stream out in qt order; the moments AllReduce is
    split in TWO chunks (qt 0-7 / 8-15) and the stats + normalize +
    store of the first chunk overlap the attention tail of the second.
    This removes the serial AllReduce + LayerNorm tail (which also ran
    at a throttled clock after the engines went idle).
"""

import os
import numpy as np
import ml_dtypes

import concourse.bacc as bacc
import concourse.bass as bass
import concourse.tile as tile
from concourse import mybir
from concourse.bass_utils import run_bass_kernel_spmd

BF16 = mybir.dt.bfloat16
F32 = mybir.dt.float32

B, S, DM, H, HD = 4, 2048, 768, 4, 192
N_CORES = 8
SCALE = 20.0
LN_EPS = 1e-5
NEG = -1e9
P = 128
NT = S // P  # 16 token tiles
NKB = S // P  # 16 key blocks
DP = DM // 2  # 384 dims per head-pair

# per-head (dout_offset, size) pieces within the pair's 384 dims
PIECES = {0: [(0, 128), (128, 64)], 1: [(192, 64), (256, 128)]}


def _build(has_pad: bool, has_affine: bool = True, seq: int = S,
           n_cores: int = N_CORES):
    NT = seq // P
    NKB = seq // P
    nc = bacc.Bacc("TRN2", target_bir_lowering=False, debug=False,
                   num_devices=n_cores)

    xt = nc.dram_tensor("xt", [DM, seq], BF16, kind="ExternalInput")
    wqt = nc.dram_tensor("wqt", [DM, DP], BF16, kind="ExternalInput")
    wkt = nc.dram_tensor("wkt", [DM, DP], BF16, kind="ExternalInput")
    wvt = nc.dram_tensor("wvt", [DM, DP], BF16, kind="ExternalInput")
    xres = nc.dram_tensor("xres", [seq, DP], BF16, kind="ExternalInput")
    trimask = nc.dram_tensor("trimask", [P, P], F32, kind="ExternalInput")
    if has_pad:
        qmaskneg = nc.dram_tensor("qmaskneg", [seq], F32, kind="ExternalInput")
    if has_affine:
        gb = nc.dram_tensor("gb", [2, DP], F32, kind="ExternalInput")
    out = nc.dram_tensor("out", [seq, DP], F32, kind="ExternalOutput")

    groups = [[2 * i, 2 * i + 1] for i in range(max(1, n_cores // 2))]
    use_cc = os.environ.get("BASS_KERNEL_NO_CC", "0") != "1"

    with tile.TileContext(nc) as tc:
        with (
            tc.tile_pool(name="wp", bufs=1) as wp,        # weights, x^T, consts
            tc.tile_pool(name="qk", bufs=1) as qk,        # Q^T/K^T per head
            tc.tile_pool(name="vp", bufs=1) as vp,        # V f32 + V' bf16
            tc.tile_pool(name="ep", bufs=1) as ep,        # E^T strips
            tc.tile_pool(name="yp", bufs=1) as yp,        # y, xres, moments
            tc.tile_pool(name="tmp", bufs=3) as tmp,      # small rotating
            tc.tile_pool(name="mm", bufs=2, space="PSUM") as mmps,
            tc.tile_pool(name="sc", bufs=2, space="PSUM") as scps,
            tc.tile_pool(name="x1", bufs=2, space="PSUM") as x1ps,
            tc.tile_pool(name="dram", bufs=1, space="DRAM") as dram,
        ):
            # ---------------- load inputs ----------------
            xt_sb = []
            for i in range(6):
                t = wp.tile([P, seq], BF16, tag=f"xt{i}")
                nc.sync.dma_start(out=t, in_=xt[i * P:(i + 1) * P, :])
                xt_sb.append(t)
            w_sb = {}
            for name, drt in (("q", wqt), ("k", wkt), ("v", wvt)):
                t = wp.tile([P, 6, DP], BF16, tag=f"w{name}")
                nc.sync.dma_start(
                    out=t, in_=drt.rearrange("(c p) d -> p c d", p=P))
                w_sb[name] = t
            tri_sb = wp.tile([P, P], F32, tag="tri")
            nc.sync.dma_start(out=tri_sb, in_=trimask[:])
            if has_pad:
                qm_sb = wp.tile([P, seq], F32, tag="qm")
                nc.sync.dma_start(out=qm_sb, in_=bass.AP(
                    tensor=qmaskneg, offset=0, ap=[[0, P], [1, seq]]))
            eps_sb = wp.tile([P, 1], F32, tag="eps")
            nc.vector.memset(eps_sb, LN_EPS)

            if use_cc:
                ccw_in = dram.tile([P, 16], F32)
                ccw_out = dram.tile([P, 16], F32)
                wseed = tmp.tile([P, 16], F32, tag="wseed")
                nc.vector.memset(wseed, 0.0)
                nc.sync.dma_start(out=ccw_in[:], in_=wseed)
                nc.gpsimd.collective_compute(
                    "AllReduce", mybir.AluOpType.add, replica_groups=groups,
                    ins=[ccw_in.opt()], outs=[ccw_out.opt()])
            # warm the Sqrt + Exp activation tables off the critical path
            sqwarm = tmp.tile([P, 1], F32, tag="sqwarm")
            nc.scalar.activation(out=sqwarm, in_=eps_sb,
                                 func=mybir.ActivationFunctionType.Sqrt,
                                 bias=eps_sb)
            expwarm = tmp.tile([P, 1], F32, tag="expwarm")
            nc.scalar.activation(out=expwarm, in_=eps_sb,
                                 func=mybir.ActivationFunctionType.Exp,
                                 scale=1.0)

            # ---------------- QKV projections ----------------
            # Q^T/K^T per head: pieces along dout; [piece_size, S] bf16 tiles
            qt_sb = {}  # (h, piece_idx) -> tile
            kt_sb = {}
            for h in range(2):
                for pi, (o, sz) in enumerate(PIECES[h]):
                    for which, store in (("q", qt_sb), ("k", kt_sb)):
                        dst = qk.tile([sz, seq], BF16, tag=f"{which}t{h}{pi}")
                        store[(h, pi)] = dst
                        for qc in range(seq // 512):
                            ps = mmps.tile([P, 512], F32, tag="mm")
                            for c in range(6):
                                nc.tensor.matmul(
                                    ps[:sz, :],
                                    w_sb[which][:, c, o:o + sz],
                                    xt_sb[c][:, qc * 512:(qc + 1) * 512],
                                    start=(c == 0), stop=(c == 5))
                            dslice = dst[:, qc * 512:(qc + 1) * 512]
                            if which == "q":
                                nc.vector.tensor_copy(out=dslice,
                                                      in_=ps[:sz, :])
                            else:
                                nc.scalar.copy(out=dslice, in_=ps[:sz, :])

            # V: [128, NT, 384] f32 token-tiled
            v_sb = vp.tile([P, NT, DP], BF16, tag="v")
            for t in range(NT):
                ps = mmps.tile([P, 512], F32, tag="mm")
                for c in range(6):
                    nc.tensor.matmul(
                        ps[:, 0:DP],
                        xt_sb[c][:, t * P:(t + 1) * P],
                        w_sb["v"][:, c, :],
                        start=(c == 0), stop=(c == 5))
                nc.scalar.copy(out=v_sb[:, t, :], in_=ps[:, 0:DP])

            # y accumulator (x1 + residual), f32
            y_sb = yp.tile([P, NT, DP], F32, tag="y")
            xres_sb = yp.tile([P, NT, DP], BF16, tag="xres")
            nc.sync.dma_start(
                out=xres_sb, in_=xres.rearrange("(t p) d -> p t d", p=P))
            if has_affine:
                gamma_sb = wp.tile([P, DP], F32, tag="gamma")
                nc.sync.dma_start(out=gamma_sb, in_=bass.AP(
                    tensor=gb, offset=0, ap=[[0, P], [1, DP]]))
                beta_sb = wp.tile([P, DP], F32, tag="beta")
                nc.sync.dma_start(out=beta_sb, in_=bass.AP(
                    tensor=gb, offset=DP, ap=[[0, P], [1, DP]]))
            # moments: cols [0, NT) = sum(y), [NT, 2NT) = sum(y^2)
            mom = yp.tile([P, 2 * NT], F32, tag="mom")
            mu = yp.tile([P, NT], F32, tag="mu")
            rstd = yp.tile([P, NT], F32, tag="rstd")

            # ---------------- per-head scores + exp + colsums ----------------
            estrips = {}
            vprs = {}
            for h in range(2):
                pieces = PIECES[h]
                for kb in range(NKB):
                    L = seq - kb * P
                    estrip = ep.tile([P, L], BF16, tag=f"e{h}_{kb}")
                    estrips[(h, kb)] = estrip
                    accs = []
                    nch = (L + 1023) // 1024
                    for ci in range(nch):
                        c0 = ci * 1024
                        clen = min(1024, L - c0)
                        ps = scps.tile([P, 1024], F32, tag="sc")
                        for sub in range((clen + 511) // 512):
                            s0 = sub * 512
                            n = min(512, clen - s0)
                            for pi, (o, sz) in enumerate(pieces):
                                nc.tensor.matmul(
                                    ps[:, s0:s0 + n],
                                    kt_sb[(h, pi)][:, kb * P:(kb + 1) * P],
                                    qt_sb[(h, pi)][:, kb * P + c0 + s0:
                                                   kb * P + c0 + s0 + n],
                                    start=(pi == 0), stop=(pi == len(pieces) - 1))
                        if ci == 0:
                            nc.vector.tensor_add(
                                out=ps[:, 0:P], in0=ps[:, 0:P], in1=tri_sb)
                        if has_pad:
                            nc.vector.tensor_add(
                                out=ps[:, 0:clen], in0=ps[:, 0:clen],
                                in1=qm_sb[:, kb * P + c0:kb * P + c0 + clen])
                        acc = tmp.tile([P, 1], F32, tag="acc")
                        nc.scalar.activation(
                            out=estrip[:, c0:c0 + clen], in_=ps[:, 0:clen],
                            func=mybir.ActivationFunctionType.Exp,
                            scale=1.0 / SCALE, accum_out=acc)
                        accs.append(acc)
                    if len(accs) == 2:
                        csum = tmp.tile([P, 1], F32, tag="csum")
                        nc.vector.tensor_add(out=csum, in0=accs[0], in1=accs[1])
                    else:
                        csum = accs[0]
                    rc = tmp.tile([P, 1], F32, tag="rc")
                    nc.vector.reciprocal(out=rc, in_=csum)
                    vpr = vp.tile([P, HD], BF16, tag=f"vp{h}_{kb}")
                    nc.vector.tensor_scalar_mul(
                        out=vpr, in0=v_sb[:, kb, h * HD:(h + 1) * HD],
                        scalar1=rc)
                    vprs[(h, kb)] = vpr

            # ---------------- chunked LN: AllReduce + stats + store ----------
            def emit_chunk_ln(ck):
                q0 = 8 * ck
                msum = yp.tile([P, 16], F32, tag=f"msum{ck}")
                if use_cc:
                    cc_in = dram.tile([P, 16], F32, tag=f"cci{ck}")
                    cc_out = dram.tile([P, 16], F32, tag=f"cco{ck}")
                    nc.sync.dma_start(out=cc_in[:, 0:8],
                                      in_=mom[:, q0:q0 + 8])
                    nc.sync.dma_start(out=cc_in[:, 8:16],
                                      in_=mom[:, NT + q0:NT + q0 + 8])
                    nc.gpsimd.collective_compute(
                        "AllReduce", mybir.AluOpType.add,
                        replica_groups=groups,
                        ins=[cc_in.opt()], outs=[cc_out.opt()])
                    nc.sync.dma_start(out=msum, in_=cc_out[:])
                else:
                    nc.vector.tensor_scalar_mul(
                        out=msum[:, 0:8], in0=mom[:, q0:q0 + 8], scalar1=2.0)
                    nc.vector.tensor_scalar_mul(
                        out=msum[:, 8:16], in0=mom[:, NT + q0:NT + q0 + 8],
                        scalar1=2.0)
                nc.scalar.mul(out=mu[:, q0:q0 + 8], in_=msum[:, 0:8],
                              mul=1.0 / DM)
                musq = tmp.tile([P, 8], F32, tag="musq")
                nc.vector.tensor_mul(out=musq, in0=mu[:, q0:q0 + 8],
                                     in1=mu[:, q0:q0 + 8])
                var8 = tmp.tile([P, 8], F32, tag="var8")
                nc.vector.scalar_tensor_tensor(
                    out=var8, in0=msum[:, 8:16], scalar=1.0 / DM, in1=musq,
                    op0=mybir.AluOpType.mult, op1=mybir.AluOpType.subtract)
                std8 = tmp.tile([P, 8], F32, tag="std8")
                nc.scalar.activation(out=std8, in_=var8,
                                     func=mybir.ActivationFunctionType.Sqrt,
                                     bias=eps_sb)
                nc.vector.reciprocal(out=rstd[:, q0:q0 + 8], in_=std8)
                for qt in range(q0, q0 + 8):
                    if has_affine:
                        z = tmp.tile([P, DP], F32, tag="z")
                        nc.vector.scalar_tensor_tensor(
                            out=z, in0=y_sb[:, qt, :],
                            scalar=mu[:, qt:qt + 1], in1=gamma_sb,
                            op0=mybir.AluOpType.subtract,
                            op1=mybir.AluOpType.mult)
                        nc.vector.scalar_tensor_tensor(
                            out=y_sb[:, qt, :], in0=z,
                            scalar=rstd[:, qt:qt + 1], in1=beta_sb,
                            op0=mybir.AluOpType.mult,
                            op1=mybir.AluOpType.add)
                    else:
                        eng = nc.vector if qt % 2 == 0 else nc.gpsimd
                        eng.tensor_scalar(
                            out=y_sb[:, qt, :], in0=y_sb[:, qt, :],
                            scalar1=mu[:, qt:qt + 1],
                            scalar2=rstd[:, qt:qt + 1],
                            op0=mybir.AluOpType.subtract,
                            op1=mybir.AluOpType.mult)
                    nc.sync.dma_start(
                        out=out[qt * P:(qt + 1) * P, :], in_=y_sb[:, qt, :])

            # ---------------- x1 for BOTH heads per query tile ----------------
            for qt in range(NT):
                for h in range(2):
                    ps = x1ps.tile([P, HD], F32, tag="x1")
                    for kb in range(qt + 1):
                        nc.tensor.matmul(
                            ps,
                            estrips[(h, kb)][:, (qt - kb) * P:(qt - kb + 1) * P],
                            vprs[(h, kb)],
                            start=(kb == 0), stop=(kb == qt))
                    nc.vector.tensor_add(
                        out=y_sb[:, qt, h * HD:(h + 1) * HD],
                        in0=ps,
                        in1=xres_sb[:, qt, h * HD:(h + 1) * HD])
                nc.vector.reduce_sum(
                    out=mom[:, qt:qt + 1], in_=y_sb[:, qt, :],
                    axis=mybir.AxisListType.X, op=mybir.AluOpType.add)
                sq = tmp.tile([P, DP], F32, tag="sq")
                nc.scalar.activation(
                    out=sq, in_=y_sb[:, qt, :],
                    func=mybir.ActivationFunctionType.Square,
                    accum_out=mom[:, NT + qt:NT + qt + 1])
                if qt == 7:
                    emit_chunk_ln(0)
            emit_chunk_ln(1)

    nc.compile()
    return nc


_NC_CACHE = {}


def _get_nc(key):
    if key not in _NC_CACHE:
        _NC_CACHE[key] = _build(*key)
    return _NC_CACHE[key]


def _install_ntff_hook():
    """Optional: enables trace=True under axon (used by test.py via env)."""
    import sys, types, contextlib, ctypes
    if "antenv.axon_hooks" in sys.modules:
        return
    so_path = "/opt/axon/libaxon_pjrt.so"
    try:
        lib = ctypes.CDLL(so_path)
    except OSError:
        return
    if not hasattr(lib, "axon_start_nrt_profile"):
        return
    lib.axon_start_nrt_profile.argtypes = [ctypes.POINTER(ctypes.c_int64),
                                           ctypes.c_size_t]
    lib.axon_start_nrt_profile.restype = ctypes.c_int64
    lib.axon_stop_nrt_profile.argtypes = [ctypes.c_char_p]
    lib.axon_stop_nrt_profile.restype = ctypes.c_int64

    @contextlib.contextmanager
    def _hook(output_dir, device_ids):
        import jax
        jax.devices()
        if device_ids:
            ids = (ctypes.c_int64 * len(device_ids))(*device_ids)
            rc = lib.axon_start_nrt_profile(ids, len(device_ids))
        else:
            rc = lib.axon_start_nrt_profile(None, 0)
        if rc != 0:
            raise RuntimeError(f"axon_start_nrt_profile rc={rc}")
        try:
            yield
        finally:
            lib.axon_stop_nrt_profile(str(output_dir).encode())

    mod = types.ModuleType("antenv.axon_hooks")
    mod.get_axon_ntff_profile_hook = lambda: _hook
    mod.set_axon_ntff_profile_hook = lambda h: None
    sys.modules["antenv.axon_hooks"] = mod
    import concourse.bass_utils as bu
    bu.upload_artifacts = lambda tmpdir: "local://skipped"


def make_in_maps(x, attention_mask, Wq, Wk, Wv, gamma, beta,
                 n_cores=N_CORES):
    x = np.asarray(x, dtype=np.float32)
    has_pad = bool(np.asarray(attention_mask).any())
    has_affine = not (np.all(np.asarray(gamma) == 1.0)
                      and np.all(np.asarray(beta) == 0.0))

    ii, jj = np.meshgrid(np.arange(P), np.arange(P), indexing="ij")
    trimask = np.where(jj >= ii, 0.0, NEG).astype(np.float32)

    in_maps = []
    for c in range(n_cores):
        b, p = c // 2, c % 2
        po = p * DP
        m = {
            "xt": np.ascontiguousarray(x[b].T).astype(ml_dtypes.bfloat16),
            "wqt": np.ascontiguousarray(
                np.asarray(Wq, np.float32)[po:po + DP, :].T).astype(
                    ml_dtypes.bfloat16),
            "wkt": np.ascontiguousarray(
                np.asarray(Wk, np.float32)[po:po + DP, :].T).astype(
                    ml_dtypes.bfloat16),
            "wvt": np.ascontiguousarray(
                np.asarray(Wv, np.float32)[po:po + DP, :].T).astype(
                    ml_dtypes.bfloat16),
            "xres": np.ascontiguousarray(x[b][:, po:po + DP]).astype(
                ml_dtypes.bfloat16),
            "trimask": trimask,
        }
        if has_pad:
            m["qmaskneg"] = np.where(
                attention_mask[b, 0], NEG, 0.0).astype(np.float32)
        if has_affine:
            m["gb"] = np.stack([np.asarray(gamma, np.float32)[po:po + DP],
                                np.asarray(beta, np.float32)[po:po + DP]])
        in_maps.append(m)

    key = (has_pad, has_affine, S, n_cores)
    return in_maps, key


_LAST_RESULT = None  # BassKernelResults of the last run (for test harness)


def kernel(x, attention_mask, Wq, bq, Wk, bk, Wv, bv, gamma, beta):
    has_bias = bool(np.any(np.asarray(bq)) or np.any(np.asarray(bk))
                    or np.any(np.asarray(bv)))
    assert not has_bias, "nonzero qkv bias path not implemented"

    trace = os.environ.get("BASS_KERNEL_TRACE", "0") == "1"
    if trace:
        _install_ntff_hook()

    in_maps, key = make_in_maps(x, attention_mask, Wq, Wk, Wv, gamma, beta)
    nc = _get_nc(key)
    res = run_bass_kernel_spmd(nc, in_maps, core_ids=list(range(N_CORES)),
                               trace=trace)
    global _LAST_RESULT
    _LAST_RESULT = res

    out = np.empty((B, S, DM), dtype=np.float32)
    for c in range(N_CORES):
        b, p = c // 2, c % 2
        out[b, :, p * DP:(p + 1) * DP] = res.results[c]["out"]
    return out


# revision 11
# speedup vs baseline: 1.2116x; 1.1990x over previous
"""Trainium2 Bass kernel for nn_MultiHeadAttention (B=4, S=2048, d_model=768, H=4).

Reference semantics (note the unusual softmax over the QUERY axis):
    Q = x @ Wq.T + bq ; K, V likewise
    w = Q K^T / 20 ; causal (strict upper triangle) + padding mask -> -1e9
    attn = softmax(w, axis=QUERY)          # column softmax
    x1 = attn @ V ; y = x + concat_heads(x1) ; out = LayerNorm(y) * gamma + beta

Sharding (8 cores): core c handles batch b=c//2 and head-pair p=c%2
(heads {2p, 2p+1} = model dims [384p, 384p+384)).  Attention per (b, head)
is fully local.  The only cross-core coupling is LayerNorm's per-token
mean/var over all 768 dims -> a pair-AllReduce of partial moments.

Perf structure vs the naive version:
  * Scores are built TRANSPOSED, S^T[k, q] (keys on partitions), so the
    column-softmax denominator c[k] = sum_q exp(S/20) falls out of the
    exp's accum_out for free, and E^T strips are directly the lhsT of
    the attn @ V matmul.  1/c is folded into V rows.
  * x1 for BOTH heads is computed per query-tile (interleaved heads),
    so the LN moments stream out in qt order; the moments AllReduce is
    split in TWO chunks (qt 0-7 / 8-15) and the stats + normalize +
    store of the first chunk overlap the attention tail of the second.
    This removes the serial AllReduce + LayerNorm tail (which also ran
    at a throttled clock after the engines went idle).
"""

import os
import numpy as np
import ml_dtypes

import concourse.bacc as bacc
import concourse.bass as bass
import concourse.tile as tile
from concourse import mybir
from concourse.bass_utils import run_bass_kernel_spmd

BF16 = mybir.dt.bfloat16
F32 = mybir.dt.float32

B, S, DM, H, HD = 4, 2048, 768, 4, 192
N_CORES = 8
SCALE = 20.0
LN_EPS = 1e-5
NEG = -1e9
P = 128
NT = S // P  # 16 token tiles
NKB = S // P  # 16 key blocks
DP = DM // 2  # 384 dims per head-pair

# per-head (dout_offset, size) pieces within the pair's 384 dims
PIECES = {0: [(0, 128), (128, 64)], 1: [(192, 64), (256, 128)]}


def _build(has_pad: bool, has_affine: bool = True, seq: int = S,
           n_cores: int = N_CORES):
    NT = seq // P
    NKB = seq // P
    nc = bacc.Bacc("TRN2", target_bir_lowering=False, debug=False,
                   num_devices=n_cores)

    xt = nc.dram_tensor("xt", [DM, seq], BF16, kind="ExternalInput")
    wqt = nc.dram_tensor("wqt", [DM, DP], BF16, kind="ExternalInput")
    wkt = nc.dram_tensor("wkt", [DM, DP], BF16, kind="ExternalInput")
    wvt = nc.dram_tensor("wvt", [DM, DP], BF16, kind="ExternalInput")
    xres = nc.dram_tensor("xres", [seq, DP], BF16, kind="ExternalInput")
    trimask = nc.dram_tensor("trimask", [P, P], F32, kind="ExternalInput")
    if has_pad:
        qmaskneg = nc.dram_tensor("qmaskneg", [seq], F32, kind="ExternalInput")
    if has_affine:
        gb = nc.dram_tensor("gb", [2, DP], F32, kind="ExternalInput")
    out = nc.dram_tensor("out", [seq, DP], F32, kind="ExternalOutput")

    groups = [[2 * i, 2 * i + 1] for i in range(max(1, n_cores // 2))]
    use_cc = os.environ.get("BASS_KERNEL_NO_CC", "0") != "1"

    with tile.TileContext(nc) as tc:
        with (
            tc.tile_pool(name="wp", bufs=1) as wp,        # weights, x^T, consts
            tc.tile_pool(name="qk", bufs=1) as qk,        # Q^T/K^T per head
            tc.tile_pool(name="vp", bufs=1) as vp,        # V f32 + V' bf16
            tc.tile_pool(name="ep", bufs=1) as ep,        # E^T strips
            tc.tile_pool(name="yp", bufs=1) as yp,        # y, xres, moments
            tc.tile_pool(name="tmp", bufs=3) as tmp,      # small rotating
            tc.tile_pool(name="mm", bufs=2, space="PSUM") as mmps,
            tc.tile_pool(name="sc", bufs=2, space="PSUM") as scps,
            tc.tile_pool(name="x1", bufs=2, space="PSUM") as x1ps,
            tc.tile_pool(name="dram", bufs=1, space="DRAM") as dram,
        ):
            # ---------------- load inputs ----------------
            xt_sb = []
            for i in range(6):
                t = wp.tile([P, seq], BF16, tag=f"xt{i}")
                nc.sync.dma_start(out=t, in_=xt[i * P:(i + 1) * P, :])
                xt_sb.append(t)
            w_sb = {}
            for name, drt in (("q", wqt), ("k", wkt), ("v", wvt)):
                t = wp.tile([P, 6, DP], BF16, tag=f"w{name}")
                nc.sync.dma_start(
                    out=t, in_=drt.rearrange("(c p) d -> p c d", p=P))
                w_sb[name] = t
            tri_sb = wp.tile([P, P], F32, tag="tri")
            nc.sync.dma_start(out=tri_sb, in_=trimask[:])
            if has_pad:
                qm_sb = wp.tile([P, seq], F32, tag="qm")
                nc.sync.dma_start(out=qm_sb, in_=bass.AP(
                    tensor=qmaskneg, offset=0, ap=[[0, P], [1, seq]]))
            eps_sb = wp.tile([P, 1], F32, tag="eps")
            nc.vector.memset(eps_sb, LN_EPS)

            if use_cc:
                ccw_in = dram.tile([P, 16], F32)
                ccw_out = dram.tile([P, 16], F32)
                wseed = tmp.tile([P, 16], F32, tag="wseed")
                nc.vector.memset(wseed, 0.0)
                nc.sync.dma_start(out=ccw_in[:], in_=wseed)
                nc.gpsimd.collective_compute(
                    "AllReduce", mybir.AluOpType.add, replica_groups=groups,
                    ins=[ccw_in.opt()], outs=[ccw_out.opt()])
            # warm the Sqrt + Exp activation tables off the critical path
            sqwarm = tmp.tile([P, 1], F32, tag="sqwarm")
            nc.scalar.activation(out=sqwarm, in_=eps_sb,
                                 func=mybir.ActivationFunctionType.Sqrt,
                                 bias=eps_sb)
            expwarm = tmp.tile([P, 1], F32, tag="expwarm")
            nc.scalar.activation(out=expwarm, in_=eps_sb,
                                 func=mybir.ActivationFunctionType.Exp,
                                 scale=1.0)

            # ---------------- QKV projections ----------------
            # Q^T/K^T per head: pieces along dout; [piece_size, S] bf16 tiles
            qt_sb = {}  # (h, piece_idx) -> tile
            kt_sb = {}
            for h in range(2):
                for pi, (o, sz) in enumerate(PIECES[h]):
                    for which, store in (("q", qt_sb), ("k", kt_sb)):
                        dst = qk.tile([sz, seq], BF16, tag=f"{which}t{h}{pi}")
                        store[(h, pi)] = dst
                        for qc in range(seq // 512):
                            ps = mmps.tile([P, 512], F32, tag="mm")
                            for c in range(6):
                                nc.tensor.matmul(
                                    ps[:sz, :],
                                    w_sb[which][:, c, o:o + sz],
                                    xt_sb[c][:, qc * 512:(qc + 1) * 512],
                                    start=(c == 0), stop=(c == 5))
                            dslice = dst[:, qc * 512:(qc + 1) * 512]
                            if which == "q":
                                nc.vector.tensor_copy(out=dslice,
                                                      in_=ps[:sz, :])
                            else:
                                nc.scalar.copy(out=dslice, in_=ps[:sz, :])

            # V: [128, NT, 384] f32 token-tiled
            v_sb = vp.tile([P, NT, DP], BF16, tag="v")
            for t in range(NT):
                ps = mmps.tile([P, 512], F32, tag="mm")
                for c in range(6):
                    nc.tensor.matmul(
                        ps[:, 0:DP],
                        xt_sb[c][:, t * P:(t + 1) * P],
                        w_sb["v"][:, c, :],
                        start=(c == 0), stop=(c == 5))
                nc.scalar.copy(out=v_sb[:, t, :], in_=ps[:, 0:DP])

            # y accumulator (x1 + residual), f32
            y_sb = yp.tile([P, NT, DP], F32, tag="y")
            xres_sb = yp.tile([P, NT, DP], BF16, tag="xres")
            nc.sync.dma_start(
                out=xres_sb, in_=xres.rearrange("(t p) d -> p t d", p=P))
            if has_affine:
                gamma_sb = wp.tile([P, DP], F32, tag="gamma")
                nc.sync.dma_start(out=gamma_sb, in_=bass.AP(
                    tensor=gb, offset=0, ap=[[0, P], [1, DP]]))
                beta_sb = wp.tile([P, DP], F32, tag="beta")
                nc.sync.dma_start(out=beta_sb, in_=bass.AP(
                    tensor=gb, offset=DP, ap=[[0, P], [1, DP]]))
            # moments: cols [0, NT) = sum(y), [NT, 2NT) = sum(y^2)
            mom = yp.tile([P, 2 * NT], F32, tag="mom")
            mu = yp.tile([P, NT], F32, tag="mu")
            rstd = yp.tile([P, NT], F32, tag="rstd")

            # ---------------- scores + exp + colsums (kb-major) ----------------
            estrips = {}
            vprs = {}

            def emit_scores(h, kb):
                pieces = PIECES[h]
                L = seq - kb * P
                estrip = ep.tile([P, L], BF16, tag=f"e{h}_{kb}")
                estrips[(h, kb)] = estrip
                accs = []
                nch = (L + 1023) // 1024
                for ci in range(nch):
                    c0 = ci * 1024
                    clen = min(1024, L - c0)
                    ps = scps.tile([P, 1024], F32, tag="sc")
                    for sub in range((clen + 511) // 512):
                        s0 = sub * 512
                        n = min(512, clen - s0)
                        for pi, (o, sz) in enumerate(pieces):
                            nc.tensor.matmul(
                                ps[:, s0:s0 + n],
                                kt_sb[(h, pi)][:, kb * P:(kb + 1) * P],
                                qt_sb[(h, pi)][:, kb * P + c0 + s0:
                                               kb * P + c0 + s0 + n],
                                start=(pi == 0), stop=(pi == len(pieces) - 1))
                    if ci == 0:
                        nc.vector.tensor_add(
                            out=ps[:, 0:P], in0=ps[:, 0:P], in1=tri_sb)
                    if has_pad:
                        nc.vector.tensor_add(
                            out=ps[:, 0:clen], in0=ps[:, 0:clen],
                            in1=qm_sb[:, kb * P + c0:kb * P + c0 + clen])
                    acc = tmp.tile([P, 1], F32, tag="acc")
                    nc.scalar.activation(
                        out=estrip[:, c0:c0 + clen], in_=ps[:, 0:clen],
                        func=mybir.ActivationFunctionType.Exp,
                        scale=1.0 / SCALE, accum_out=acc)
                    accs.append(acc)
                if len(accs) == 2:
                    csum = tmp.tile([P, 1], F32, tag="csum")
                    nc.vector.tensor_add(out=csum, in0=accs[0], in1=accs[1])
                else:
                    csum = accs[0]
                rc = tmp.tile([P, 1], F32, tag="rc")
                nc.vector.reciprocal(out=rc, in_=csum)
                vpr = vp.tile([P, HD], BF16, tag=f"vp{h}_{kb}")
                nc.vector.tensor_scalar_mul(
                    out=vpr, in0=v_sb[:, kb, h * HD:(h + 1) * HD],
                    scalar1=rc)
                vprs[(h, kb)] = vpr

            # ---------------- chunked LN: AllReduce + stats + store ----------
            def emit_chunk_ln(ck):
                q0 = 8 * ck
                msum = yp.tile([P, 16], F32, tag=f"msum{ck}")
                if use_cc:
                    cc_in = dram.tile([P, 16], F32, tag=f"cci{ck}")
                    cc_out = dram.tile([P, 16], F32, tag=f"cco{ck}")
                    nc.sync.dma_start(out=cc_in[:, 0:8],
                                      in_=mom[:, q0:q0 + 8])
                    nc.sync.dma_start(out=cc_in[:, 8:16],
                                      in_=mom[:, NT + q0:NT + q0 + 8])
                    nc.gpsimd.collective_compute(
                        "AllReduce", mybir.AluOpType.add,
                        replica_groups=groups,
                        ins=[cc_in.opt()], outs=[cc_out.opt()])
                    nc.sync.dma_start(out=msum, in_=cc_out[:])
                else:
                    nc.vector.tensor_scalar_mul(
                        out=msum[:, 0:8], in0=mom[:, q0:q0 + 8], scalar1=2.0)
                    nc.vector.tensor_scalar_mul(
                        out=msum[:, 8:16], in0=mom[:, NT + q0:NT + q0 + 8],
                        scalar1=2.0)
                nc.scalar.mul(out=mu[:, q0:q0 + 8], in_=msum[:, 0:8],
                              mul=1.0 / DM)
                musq = tmp.tile([P, 8], F32, tag="musq")
                nc.vector.tensor_mul(out=musq, in0=mu[:, q0:q0 + 8],
                                     in1=mu[:, q0:q0 + 8])
                var8 = tmp.tile([P, 8], F32, tag="var8")
                nc.vector.scalar_tensor_tensor(
                    out=var8, in0=msum[:, 8:16], scalar=1.0 / DM, in1=musq,
                    op0=mybir.AluOpType.mult, op1=mybir.AluOpType.subtract)
                std8 = tmp.tile([P, 8], F32, tag="std8")
                nc.scalar.activation(out=std8, in_=var8,
                                     func=mybir.ActivationFunctionType.Sqrt,
                                     bias=eps_sb)
                nc.vector.reciprocal(out=rstd[:, q0:q0 + 8], in_=std8)
                for qt in range(q0, q0 + 8):
                    if has_affine:
                        z = tmp.tile([P, DP], F32, tag="z")
                        nc.vector.scalar_tensor_tensor(
                            out=z, in0=y_sb[:, qt, :],
                            scalar=mu[:, qt:qt + 1], in1=gamma_sb,
                            op0=mybir.AluOpType.subtract,
                            op1=mybir.AluOpType.mult)
                        nc.vector.scalar_tensor_tensor(
                            out=y_sb[:, qt, :], in0=z,
                            scalar=rstd[:, qt:qt + 1], in1=beta_sb,
                            op0=mybir.AluOpType.mult,
                            op1=mybir.AluOpType.add)
                    else:
                        nc.vector.tensor_scalar(
                            out=y_sb[:, qt, :], in0=y_sb[:, qt, :],
                            scalar1=mu[:, qt:qt + 1],
                            scalar2=rstd[:, qt:qt + 1],
                            op0=mybir.AluOpType.subtract,
                            op1=mybir.AluOpType.mult)
                    nc.sync.dma_start(
                        out=out[qt * P:(qt + 1) * P, :], in_=y_sb[:, qt, :])

            # -------- kb-major: scores for both heads, then x1[qt=kb] --------
            # x1[qt] needs E strips kb<=qt of BOTH heads, which are exactly
            # the ones emitted by the time scores kb=qt are done -> moments
            # stream in qt order and chunk-A LN overlaps the attention tail.
            for kb in range(NKB):
                emit_scores(0, kb)
                emit_scores(1, kb)
                qt = kb
                for h in range(2):
                    ps = x1ps.tile([P, HD], F32, tag="x1")
                    for k2 in range(qt + 1):
                        nc.tensor.matmul(
                            ps,
                            estrips[(h, k2)][:, (qt - k2) * P:(qt - k2 + 1) * P],
                            vprs[(h, k2)],
                            start=(k2 == 0), stop=(k2 == qt))
                    nc.vector.tensor_add(
                        out=y_sb[:, qt, h * HD:(h + 1) * HD],
                        in0=ps,
                        in1=xres_sb[:, qt, h * HD:(h + 1) * HD])
                nc.vector.reduce_sum(
                    out=mom[:, qt:qt + 1], in_=y_sb[:, qt, :],
                    axis=mybir.AxisListType.X, op=mybir.AluOpType.add)
                sq = tmp.tile([P, DP], F32, tag="sq")
                nc.scalar.activation(
                    out=sq, in_=y_sb[:, qt, :],
                    func=mybir.ActivationFunctionType.Square,
                    accum_out=mom[:, NT + qt:NT + qt + 1])
                if qt == 7:
                    emit_chunk_ln(0)
            emit_chunk_ln(1)

            # hold the PE (and package clocks) busy through the chunk-B
            # AllReduce + normalize tail: harmless matmuls into a scratch
            # PSUM bank, emitted after all real work so they run last.
            for j in range(48):
                jp = mmps.tile([P, 512], F32, tag="mm")
                nc.tensor.matmul(jp, xt_sb[0][:, 0:P], xt_sb[1][:, 0:512],
                                 start=True, stop=True)

    nc.compile()
    return nc


_NC_CACHE = {}


def _get_nc(key):
    if key not in _NC_CACHE:
        _NC_CACHE[key] = _build(*key)
    return _NC_CACHE[key]


def _install_ntff_hook():
    """Optional: enables trace=True under axon (used by test.py via env)."""
    import sys, types, contextlib, ctypes
    if "antenv.axon_hooks" in sys.modules:
        return
    so_path = "/opt/axon/libaxon_pjrt.so"
    try:
        lib = ctypes.CDLL(so_path)
    except OSError:
        return
    if not hasattr(lib, "axon_start_nrt_profile"):
        return
    lib.axon_start_nrt_profile.argtypes = [ctypes.POINTER(ctypes.c_int64),
                                           ctypes.c_size_t]
    lib.axon_start_nrt_profile.restype = ctypes.c_int64
    lib.axon_stop_nrt_profile.argtypes = [ctypes.c_char_p]
    lib.axon_stop_nrt_profile.restype = ctypes.c_int64

    @contextlib.contextmanager
    def _hook(output_dir, device_ids):
        import jax
        jax.devices()
        if device_ids:
            ids = (ctypes.c_int64 * len(device_ids))(*device_ids)
            rc = lib.axon_start_nrt_profile(ids, len(device_ids))
        else:
            rc = lib.axon_start_nrt_profile(None, 0)
        if rc != 0:
            raise RuntimeError(f"axon_start_nrt_profile rc={rc}")
        try:
            yield
        finally:
            lib.axon_stop_nrt_profile(str(output_dir).encode())

    mod = types.ModuleType("antenv.axon_hooks")
    mod.get_axon_ntff_profile_hook = lambda: _hook
    mod.set_axon_ntff_profile_hook = lambda h: None
    sys.modules["antenv.axon_hooks"] = mod
    import concourse.bass_utils as bu
    bu.upload_artifacts = lambda tmpdir: "local://skipped"


def make_in_maps(x, attention_mask, Wq, Wk, Wv, gamma, beta,
                 n_cores=N_CORES):
    x = np.asarray(x, dtype=np.float32)
    has_pad = bool(np.asarray(attention_mask).any())
    has_affine = not (np.all(np.asarray(gamma) == 1.0)
                      and np.all(np.asarray(beta) == 0.0))

    ii, jj = np.meshgrid(np.arange(P), np.arange(P), indexing="ij")
    trimask = np.where(jj >= ii, 0.0, NEG).astype(np.float32)

    in_maps = []
    for c in range(n_cores):
        b, p = c // 2, c % 2
        po = p * DP
        m = {
            "xt": np.ascontiguousarray(x[b].T).astype(ml_dtypes.bfloat16),
            "wqt": np.ascontiguousarray(
                np.asarray(Wq, np.float32)[po:po + DP, :].T).astype(
                    ml_dtypes.bfloat16),
            "wkt": np.ascontiguousarray(
                np.asarray(Wk, np.float32)[po:po + DP, :].T).astype(
                    ml_dtypes.bfloat16),
            "wvt": np.ascontiguousarray(
                np.asarray(Wv, np.float32)[po:po + DP, :].T).astype(
                    ml_dtypes.bfloat16),
            "xres": np.ascontiguousarray(x[b][:, po:po + DP]).astype(
                ml_dtypes.bfloat16),
            "trimask": trimask,
        }
        if has_pad:
            m["qmaskneg"] = np.where(
                attention_mask[b, 0], NEG, 0.0).astype(np.float32)
        if has_affine:
            m["gb"] = np.stack([np.asarray(gamma, np.float32)[po:po + DP],
                                np.asarray(beta, np.float32)[po:po + DP]])
        in_maps.append(m)

    key = (has_pad, has_affine, S, n_cores)
    return in_maps, key


_LAST_RESULT = None  # BassKernelResults of the last run (for test harness)


def kernel(x, attention_mask, Wq, bq, Wk, bk, Wv, bv, gamma, beta):
    has_bias = bool(np.any(np.asarray(bq)) or np.any(np.asarray(bk))
                    or np.any(np.asarray(bv)))
    assert not has_bias, "nonzero qkv bias path not implemented"

    trace = os.environ.get("BASS_KERNEL_TRACE", "0") == "1"
    if trace:
        _install_ntff_hook()

    in_maps, key = make_in_maps(x, attention_mask, Wq, Wk, Wv, gamma, beta)
    nc = _get_nc(key)
    res = run_bass_kernel_spmd(nc, in_maps, core_ids=list(range(N_CORES)),
                               trace=trace)
    global _LAST_RESULT
    _LAST_RESULT = res

    out = np.empty((B, S, DM), dtype=np.float32)
    for c in range(N_CORES):
        b, p = c // 2, c % 2
        out[b, :, p * DP:(p + 1) * DP] = res.results[c]["out"]
    return out
